# revision 60
# baseline (speedup 1.0000x reference)
"""Trainium2 Bass kernel for the SCON linear-SDE particle scan.

Reference computation: x_{t+1} = (I + DT*W_{t+1}) x_t + DT*b_{t+1} + ds*eps_t
over 10000 steps for B=512 particles with a 3-dim state, observed every 50
steps through a [4,3] projection -> loc_y [512, 201, 4].

The transition matrices depend only on theta (14 scalars), so the whole scan
is a linear map of (x0, eps).  On the host (float64) we precompute hierarchical
propagator weights that turn the scan into three levels of PE matmuls over the
noise tensor:

  level A: chunks of 10 steps   U10[c]  = sum_t  S10[c,t] ds eps_t   (1000)
  level B: windows of 50 steps  U50[w]  = sum_g  S50[w,g] U10[5w+g]  (200)
  level C: obs propagation + projection + x0/deterministic affine part

B is sharded 64 particles per core across 8 cores (pure data parallel).
Everything is shipped and computed in bf16 (noise is 99% of the output L2;
bf16 keeps rel err ~4e-3 against the 2e-2 gate, and halves both the HBM
stream and the PE column count vs f32).  Per-core device work: stream
~5.7 MB (noise as lhsT-ready [128, 250*64] bf16 tiles + weights),
250 + 80 + 11 matmuls, write [64, 804] bf16 output (host casts to f32).

Schedule notes (measured on trn2, exec ~37us from ~60us baseline):
- eps streams on the Sync HWDGE ring in 6 slices aligned to the level-A
  PSUM big-groups; weights stream on the Scalar ring.  Rates are capped
  by the 16 shared SDMA engines (~420 GB/s), so transfer time is purely
  bytes-bound; slice order = consumption order.
- level-B slots are emitted one big-group AFTER their u10 data so the PE
  (strict in-order queue) never head-of-line blocks on an in-flight
  Vector evacuation.
- level-C half 0 (obs 0..100, windows wt<=2 only) completes and DMAs out
  mid-kernel; half 1's wt3/wt4 tail runs right behind the last eps bytes.
- the measured window includes a fixed ~9us NEFF epilogue (254 semaphore
  resets + barriers) and excludes ~6us of iram loads at the front.
"""

import numpy as np
import ml_dtypes

BF16 = ml_dtypes.bfloat16

# ---------------------------------------------------------------- constants
T_TOT = 1000.0
DT = 0.1
N = 10001
TEMP_REF = 283.0
TEMP_RISE = 5.0
GAS_R = 0.008314
NSTEP = N - 1            # 10000
B = 512
NCORE = 8
BC = B // NCORE          # 64 particles per core

L1 = 10                  # level-A chunk length (steps)
NC1 = NSTEP // L1        # 1000 chunks
CPW = 5                  # chunks per window
NW = NC1 // CPW          # 200 windows
NOBS = NW + 1            # 201 observations
OBS_EVERY = 50

SUPER = 4                # chunks per level-A matmul
NMM_A = NC1 // SUPER     # 250 level-A matmuls
KE = 128                 # eps rows per level-A matmul (4 chunks x 32, 2 pad
                         # rows per chunk so blocks start 32-aligned)
NSUP_COL = 250           # eps columns groups (one per matmul)
NTILE_A = (NMM_A + 3) // 4   # 63 psum tiles (4 matmuls/tile, last has 2)

WPS = 10                 # windows per level-B slot (30 rows of 32)
NSLOT_B = NW // WPS      # 20 slots
NTILE_B = NSLOT_B // 4   # 5 u50 tiles
TAUS_PER_SLOT = 4        # u10 tiles touched per level-B slot

NOUT = 4 * NOBS          # 804
NH = NOUT // 2           # 402  (psum free-dim per matmul)

_program_cache = None
_last_results = None     # BassKernelResults of the most recent run (for test.py)

# Every psA tile is memset before use (see the loop).  The stale rows only
# ever multiply zero weight columns, BUT if a previous NEFF left NaN bits in
# PSUM, 0*NaN=NaN leaks through -- observed once as a transient all-NaN
# output.  The memsets run on the otherwise-idle Vector engine during the
# DMA-bound phase, so they are free.


# ------------------------------------------------------------- host math
def _forcings():
    times = np.linspace(0.0, T_TOT, N)
    temp = (TEMP_REF + TEMP_RISE * times / (80 * 24 * 365)
            + 10 * np.sin(2 * np.pi / 24 * times)
            + 10 * np.sin(2 * np.pi / (24 * 365) * times))
    I_S = 0.001 + 0.0005 * np.sin(2 * np.pi / (24 * 365) * times)
    I_D = 0.0001 + 5e-05 * np.sin(2 * np.pi / (24 * 365) * times)
    return temp, I_S, I_D


def _precompute(theta):
    """float64 propagator weights, packed into the device operand layouts."""
    theta = np.asarray(theta, np.float64)
    (kSr, kDr, kMr, EaS, EaD, EaM, aSD, aDS, aM, aMSC, uM, cS, cD, cM) = theta
    temp, I_S, I_D = _forcings()
    arr = lambda p, Ea: p * np.exp(-Ea / GAS_R * (1.0 / temp - 1.0 / TEMP_REF))
    k_S, k_D, k_M = arr(kSr, EaS), arr(kDr, EaD), arr(kMr, EaM)

    zeros = np.zeros(N)
    A0 = np.stack([-k_S, aDS * k_D, aM * aMSC * k_M])
    A1 = np.stack([aSD * k_S, -(uM + k_D), aM * (1 - aMSC) * k_M])
    A2 = np.stack([zeros, np.full(N, uM), -k_M])
    W = np.stack([A0, A1, A2]).transpose(2, 0, 1)          # [N,3,3]
    bias = np.stack([I_S, I_D, zeros], axis=1)             # [N,3]

    beta = np.clip(np.array([cS, cD, cM]), 1e-6, None)
    ds = np.sqrt(beta * DT)

    M = np.eye(3)[None] + DT * W[1:]                       # [10000,3,3]
    c = DT * bias[1:]                                      # [10000,3]

    # level A: within-chunk suffix products S10[c,tau] = M_{end}...M_{tau+1}
    Mc = M.reshape(NC1, L1, 3, 3)
    S10 = np.empty((NC1, L1, 3, 3))
    A10 = np.empty((NC1, 3, 3))
    for cI in range(NC1):
        acc = np.eye(3)
        S10[cI, L1 - 1] = acc
        for tau in range(L1 - 2, -1, -1):
            acc = acc @ Mc[cI, tau + 1]
            S10[cI, tau] = acc
        A10[cI] = S10[cI, 0] @ Mc[cI, 0]
    Gmat = (S10 * ds[None, None, None, :]).transpose(0, 1, 3, 2).reshape(NC1, 30, 3)

    # level B: within-window suffix products over chunks
    A10w = A10.reshape(NW, CPW, 3, 3)
    S50 = np.empty((NW, CPW, 3, 3))
    A50 = np.empty((NW, 3, 3))
    for w in range(NW):
        acc = np.eye(3)
        S50[w, CPW - 1] = acc
        for g in range(CPW - 2, -1, -1):
            acc = acc @ A10w[w, g + 1]
            S50[w, g] = acc
        A50[w] = S50[w, 0] @ A10w[w, 0]
    Hmat = S50.transpose(0, 1, 3, 2).reshape(NW, 3 * CPW, 3)   # [w, 3g+j, i]

    # deterministic trajectory at obs points (exact, float64)
    xd = np.zeros(3)
    detx = np.zeros((NOBS, 3))
    for t in range(NSTEP):
        xd = M[t] @ xd + c[t]
        if (t + 1) % OBS_EVERY == 0:
            detx[(t + 1) // OBS_EVERY] = xd

    # observation weights
    sub = np.arange(NOBS) * OBS_EVERY
    C1 = np.stack([(1 - aSD) * k_S[sub], (1 - aDS) * k_D[sub], (1 - aM) * k_M[sub]],
                  axis=1)
    Wobs = np.concatenate([np.broadcast_to(np.eye(3), (NOBS, 3, 3)),
                           C1[:, None, :]], axis=1)        # [NOBS,4,3]

    # level C: Rmat[(w,j),(n,o)] = sum_i Wobs[n,o,i] PhiW[n,w+1][i,j] (w < n)
    Rmat = np.zeros((3 * NW, NOUT))
    RX = np.zeros((3, NOUT))
    base = np.zeros(NOUT)
    for n in range(NOBS):
        WP = Wobs[n]
        base[4 * n:4 * n + 4] = WP @ detx[n]
        acc = WP.copy()
        for w in range(n - 1, -1, -1):
            Rmat[3 * w:3 * w + 3, 4 * n:4 * n + 4] = acc.T
            acc = acc @ A50[w]
        RX[:, 4 * n:4 * n + 4] = acc.T

    # ---------------- pack into device layouts (bf16) ----------------
    # Gsb [128, 3000]: block-diag lhsT shipped ready-to-use.  (On-device
    # scatter from a dense [128,750] was tried and loses: the strided
    # 3-of-12 DVE/GpSimd copies take ~3.6us each and gate every matmul.)
    # Gsb[32g+r, 12s+3g+i] = Gmat[4s+g, r, i]; rows 32g+30/31 zero pad.
    G4 = Gmat.reshape(NMM_A, SUPER, 30, 3)                 # [s,g,r,i]
    Gsb = np.zeros((SUPER, 32, NMM_A, 12), np.float32)
    for g in range(SUPER):
        Gsb[g, :30, :, 3 * g:3 * g + 3] = G4[:, g].transpose(1, 0, 2)
    Gsb = Gsb.reshape(KE, NMM_A * 12)

    # u10 row map: chunk c10, comp i -> row 32*((c10//4)%4) + 3*(c10%4) + i,
    #                                   col 64*(c10//16) + b
    # HB [128, 80*30]: level-B lhsT tiles; matmul mB = 4*slot + (tau - tau0)
    HB = np.zeros((128, NSLOT_B * TAUS_PER_SLOT * 30), np.float32)
    mB = 0
    for om in range(NSLOT_B):
        tau0 = (50 * om) // 16
        for tau in range(tau0, tau0 + TAUS_PER_SLOT):
            blk = HB[:, 30 * mB:30 * (mB + 1)]
            for rho in range(128):
                q = rho % 32
                if q >= 12:
                    continue
                c10 = 16 * tau + 4 * (rho // 32) + q // 3
                jj = q % 3
                if c10 >= NC1:
                    continue
                w = c10 // 5
                if w // WPS != om:
                    continue
                m = w - WPS * om
                g = c10 - CPW * w
                blk[rho, 3 * m:3 * m + 3] = Hmat[w, 3 * g + jj, :]
            mB += 1

    # u50 row map: window w, comp j -> row 32*((w//10)%4) + 3*(w%10) + j,
    #                                  col 64*(w//40) + b
    # Rsb: only the nonzero (triangular) column range of each (wt, half)
    # block is shipped; see _rsb_blocks() for the packing.
    blocks = _rsb_blocks()
    ncols = sum(b[3] for b in blocks)
    Rsb = np.zeros((128, ncols), np.float32)
    for wt, h, rel0, keep, off in blocks:
        blk = np.zeros((128, keep), np.float32)
        for rho in range(128):
            q = rho % 32
            if q >= 30:
                continue
            w = WPS * (4 * wt + rho // 32) + q // 3
            j = q % 3
            blk[rho] = Rmat[3 * w + j, NH * h + rel0:NH * h + rel0 + keep]
        Rsb[:, off:off + keep] = blk

    RXaug = np.concatenate([RX, base[None]], axis=0)       # [4,804]
    return dict(Gsb=Gsb.astype(BF16), HB=HB.astype(BF16),
                Rsb=Rsb.astype(BF16), RXaug=RXaug.astype(BF16))


def _rsb_blocks():
    """Nonzero column ranges of each level-C (wt, half) block.

    Window-tile wt covers windows [40wt, 40wt+40); its rows only affect
    observations n >= 40wt+1, i.e. global cols >= 4*(40wt+1).  Returns
    (wt, h, rel0, keep, packed_col_offset) for each nonempty block.
    """
    blocks = []
    off = 0
    for h in range(2):
        for wt in range(NTILE_B):
            rel0 = max(0, 4 * (40 * wt + 1) - NH * h)
            if rel0 >= NH:
                continue
            keep = NH - rel0
            blocks.append((wt, h, rel0, keep, off))
            off += keep
    return blocks


def _pack_eps(noise_core):
    """[64,10000,3] f32 -> [128, 250*64]: row 32g + (3tau+j), col 64s + b =
    eps[b, t, j] for t = 10*(4s+g) + tau; rows 32g+30, 32g+31 are zero pad."""
    a = noise_core.reshape(BC, NSTEP * 3).T          # [30000, 64] view
    a = np.ascontiguousarray(a).reshape(NSUP_COL, SUPER, 30, BC)
    out = np.zeros((SUPER, 32, NSUP_COL, BC), BF16)
    out[:, :30] = a.transpose(1, 2, 0, 3).astype(BF16)
    return out.reshape(KE, NSUP_COL * BC)


# ------------------------------------------------------------ bass program
def _build_program(**bass_kwargs):
    import concourse.bass as bass
    import concourse.tile as tile
    from concourse import bacc, mybir

    f32 = mybir.dt.float32
    bf16 = mybir.dt.bfloat16
    nc = bacc.Bacc(None, target_bir_lowering=False, **bass_kwargs)

    rblocks = _rsb_blocks()
    NRSB = sum(b[3] for b in rblocks)
    HBW = NSLOT_B * TAUS_PER_SLOT * 30          # 2400 hb columns

    eps_d = nc.dram_tensor("eps", [KE, NSUP_COL * BC], bf16, kind="ExternalInput")
    gsb_d = nc.dram_tensor("gsb", [KE, NMM_A * 12], bf16, kind="ExternalInput")
    hb_d = nc.dram_tensor("hb", [128, HBW], bf16, kind="ExternalInput")
    rsb_d = nc.dram_tensor("rsb", [128, NRSB], bf16, kind="ExternalInput")
    xr_d = nc.dram_tensor("xr", [4, BC + NOUT], bf16, kind="ExternalInput")
    out_d = nc.dram_tensor("out", [BC, NOUT], bf16, kind="ExternalOutput")

    # eps slices aligned to level-A PSUM big-group boundaries so groups
    # unblock as their slice lands; 6 slices keeps total DMA count under the
    # HWDGE ring's outstanding-dispatch depth
    SLICES = [32, 32, 64, 64, 32, 26]
    SOFF = [0, 32, 64, 128, 192, 224, 250]

    with tile.TileContext(nc) as tc:
        with (
            tc.tile_pool(name="consts", bufs=1) as consts,
            tc.tile_pool(name="epsp", bufs=1) as epsp,
            tc.tile_pool(name="psA", bufs=2, space="PSUM") as psA,
            tc.tile_pool(name="psB", bufs=1, space="PSUM") as psB,
            tc.tile_pool(name="psC", bufs=2, space="PSUM") as psC,
        ):
            gsb = consts.tile([KE, NMM_A * 12], bf16)
            hb = consts.tile([128, HBW], bf16)
            rsb = consts.tile([128, NRSB], bf16)
            xr = consts.tile([4, BC + NOUT], bf16)
            u10 = consts.tile([128, NTILE_A * BC], bf16)
            u50 = consts.tile([128, NTILE_B * BC], bf16)
            outsb = consts.tile([BC, NOUT], bf16)
            x0t = xr[:, :BC]
            rxt = xr[:, BC:]

            eps_t = []
            for i, nsup in enumerate(SLICES):
                et = epsp.tile([KE, BC * nsup], bf16, tag=f"eps{i}")
                eps_t.append(et)

            # eps streams in-order on the Sync HWDGE ring; const weights on
            # the Scalar ring.  (Measured: moving any eps slice onto the
            # scalar ring behind the big consts, or any const onto the eps
            # ring, delays the critical eps arrivals and loses 1-3us.)
            def eps_dma(eng, i):
                eng.dma_start(out=eps_t[i],
                              in_=eps_d[:, BC * SOFF[i]:BC * SOFF[i + 1]])
            # Each HWDGE ring allows ~4 un-stalled dispatches (the 5th waits
            # for the 1st's completion + ~2us HBM receipt), so dispatch count
            # per ring is kept minimal.  The final eps slice rides the scalar
            # ring, which drains ~6us before the sync ring: big7's data is
            # already resident when the A-stream reaches it, leaving eps4 as
            # the gating arrival.
            nc.scalar.dma_start(out=gsb, in_=gsb_d[:])
            for i in range(len(SLICES) - 1):
                eps_dma(nc.sync, i)
            nc.scalar.dma_start(out=hb, in_=hb_d[:])
            nc.scalar.dma_start(out=rsb, in_=rsb_d[:])
            eps_dma(nc.scalar, len(SLICES) - 1)
            nc.scalar.dma_start(out=xr, in_=xr_d[:])

            def eps_rhs(s):
                for i in range(len(SLICES)):
                    if s < SOFF[i + 1]:
                        c = s - SOFF[i]
                        return eps_t[i][:, BC * c:BC * (c + 1)]
                raise AssertionError(s)

            pb = psB.tile([128, BC * NTILE_B], f32, tag="pb")
            nc.vector.memset(pb, 0.0)

            # level-B slot om becomes runnable once level-A big group
            # ((50*om+49)//16)//8 has been evacuated to u10
            def b_dep(om):
                return ((50 * om + 49) // 16) // SG_PER_TILE

            def emit_b_slot(om):
                wt, sb = om // 4, om % 4
                tau0 = (50 * om) // 16
                for ti in range(TAUS_PER_SLOT):
                    nc.tensor.matmul(
                        pb[32 * sb:32 * sb + 30, BC * wt:BC * (wt + 1)],
                        hb[:, 30 * (4 * om + ti):30 * (4 * om + ti + 1)],
                        u10[:, BC * (tau0 + ti):BC * (tau0 + ti + 1)],
                        start=(ti == 0), stop=(ti == TAUS_PER_SLOT - 1),
                        tile_position=(0, 32 * sb),
                        skip_group_check=(om != 0 or ti != 0))

            # level-C chains; blocks for wt<=2 run early (u50[:,:3*BC] ready
            # after B slot 11), the wt>=3 tail after the final B slot
            pc = [None, None]

            def emit_c_start(h):
                # h=0 only has wt<=2 blocks, so its chain closes here
                pch = psC.tile([BC, NH], f32, tag="pc")
                pc[h] = pch
                nc.tensor.matmul(pc[h], x0t, rxt[:, NH * h:NH * (h + 1)],
                                 start=True, stop=False,
                                 skip_group_check=(h != 0))
                early = [b for b in rblocks if b[1] == h and b[0] <= 2]
                for bi, (wt, _, rel0, keep, off) in enumerate(early):
                    nc.tensor.matmul(
                        pc[h][:, rel0:rel0 + keep],
                        u50[:, BC * wt:BC * (wt + 1)],
                        rsb[:, off:off + keep],
                        start=False,
                        stop=(h == 0 and bi == len(early) - 1),
                        skip_group_check=True)

            def emit_c_block(h, want_wt, stop):
                for (wt, hh, rel0, keep, off) in rblocks:
                    if wt == want_wt and hh == h:
                        nc.tensor.matmul(
                            pc[h][:, rel0:rel0 + keep],
                            u50[:, BC * wt:BC * (wt + 1)],
                            rsb[:, off:off + keep],
                            start=False, stop=stop, skip_group_check=True)

            def emit_c_out(h):
                nc.vector.tensor_copy(outsb[:, NH * h:NH * (h + 1)], pc[h])
                nc.sync.dma_start(out=out_d[:, NH * h:NH * (h + 1)],
                                  in_=outsb[:, NH * h:NH * (h + 1)])

            # ---- level A: 250 matmuls -> u10, B/C interleaved ----
            # one PSUM bank holds 8 supergroups (8 x 64 cols = 2 KB), so one
            # DVE copy evacuates 32 matmuls' worth of output
            SG_PER_TILE = 8
            next_b = 0

            def emit_a_sg(pa, sg_lo, sgA, first):
                co = BC * (sgA - sg_lo)
                nmm = 4 if sgA < NTILE_A - 1 else NMM_A - 4 * (NTILE_A - 1)
                for sig in range(nmm):
                    s = 4 * sgA + sig
                    nc.tensor.matmul(
                        pa[32 * sig:32 * sig + 12, co:co + BC],
                        gsb[:, 12 * s:12 * (s + 1)],
                        eps_rhs(s),
                        start=True, stop=True, tile_position=(0, 32 * sig),
                        # sim's group checker mis-maps offset outs
                        skip_group_check=not (first and sig == 0))

            for big in range(8):
                sg_lo = SG_PER_TILE * big
                sg_hi = min(sg_lo + SG_PER_TILE, NTILE_A)
                pa = psA.tile([128, BC * SG_PER_TILE], f32, tag="pa")
                nc.vector.memset(pa, 0.0)
                for sgA in range(sg_lo, sg_hi):
                    emit_a_sg(pa, sg_lo, sgA, sgA == sg_lo)
                nc.vector.tensor_copy(
                    u10[:, BC * sg_lo:BC * sg_hi], pa[:, :BC * (sg_hi - sg_lo)])
                # lag B slots one big-group behind their data dependency so
                # the PE never stalls on an in-flight u10 evacuation (the
                # evac runs concurrently with the next group's A stream)
                while next_b < NSLOT_B and b_dep(next_b) <= big - 1:
                    emit_b_slot(next_b)
                    next_b += 1
                    if next_b == 12:
                        # u50 for wt 0..2 + the full h=0 chain + h=1 head can
                        # run while the remaining eps still streams in
                        nc.vector.tensor_copy(u50[:, :3 * BC], pb[:, :3 * BC])
                        emit_c_start(0)
                        emit_c_start(1)
                        emit_c_out(0)

            while next_b < NSLOT_B:
                emit_b_slot(next_b)
                next_b += 1
            nc.vector.tensor_copy(u50[:, 3 * BC:], pb[:, 3 * BC:])
            emit_c_block(1, 3, stop=False)
            emit_c_block(1, 4, stop=True)
            emit_c_out(1)

    nc.finalize()
    return nc

# ------------------------------------------------------------------ kernel
def kernel(theta, x0, noise, obs_every):
    global _program_cache, _last_results
    from concourse.bass_utils import run_bass_kernel_spmd

    assert int(obs_every) == OBS_EVERY
    theta = np.asarray(theta, np.float32)
    x0 = np.asarray(x0, np.float32)
    noise = np.asarray(noise, np.float32)

    ops = _precompute(theta.astype(np.float64))

    if _program_cache is None:
        _program_cache = _build_program()
    nc = _program_cache

    in_maps = []
    for q in range(NCORE):
        sl = slice(BC * q, BC * (q + 1))
        x0aug = np.concatenate([np.ascontiguousarray(x0[sl].T),
                                np.ones((1, BC), np.float32)], axis=0)
        xr = np.concatenate([x0aug, ops["RXaug"].astype(np.float32)],
                            axis=1).astype(BF16)
        in_maps.append({
            "eps": _pack_eps(noise[sl]),
            "gsb": ops["Gsb"],
            "hb": ops["HB"],
            "rsb": ops["Rsb"],
            "xr": xr,
        })

    import os
    trace = bool(os.environ.get("KERNEL_TRACE"))
    res = run_bass_kernel_spmd(nc, in_maps, core_ids=list(range(NCORE)),
                               trace=trace)
    _last_results = res
    out = np.concatenate(
        [np.asarray(res.results[q]["out"]).reshape(BC, NOBS, 4)
         for q in range(NCORE)], axis=0)
    return out.astype(np.float32)



# revision 61
# speedup vs baseline: 1.1323x; 1.1323x over previous
"""Trainium2 Bass kernel for the SCON linear-SDE particle scan.

Reference computation: x_{t+1} = (I + DT*W_{t+1}) x_t + DT*b_{t+1} + ds*eps_t
over 10000 steps for B=512 particles with a 3-dim state, observed every 50
steps through a [4,3] projection -> loc_y [512, 201, 4].

The transition matrices depend only on theta (14 scalars), so the whole scan
is a linear map of (x0, eps).  On the host (float64) we precompute hierarchical
propagator weights that turn the scan into three levels of PE matmuls over the
noise tensor:

  level A: chunks of 10 steps   U10[c]  = sum_t  S10[c,t] ds eps_t   (1000)
  level B: windows of 50 steps  U50[w]  = sum_g  S50[w,g] U10[5w+g]  (200)
  level C: obs propagation + projection + x0/deterministic affine part

B is sharded 64 particles per core across 8 cores (pure data parallel).
Everything is shipped and computed in bf16 (noise is 99% of the output L2;
bf16 keeps rel err ~4e-3 against the 2e-2 gate, and halves both the HBM
stream and the PE column count vs f32).  Per-core device work: stream
~5.7 MB (noise as lhsT-ready [128, 250*64] bf16 tiles + weights),
250 + 80 + 11 matmuls, write [64, 804] bf16 output (host casts to f32).

Schedule notes (measured on trn2, exec ~37us from ~60us baseline):
- eps streams on the Sync HWDGE ring in 6 slices aligned to the level-A
  PSUM big-groups; weights stream on the Scalar ring.  Rates are capped
  by the 16 shared SDMA engines (~420 GB/s), so transfer time is purely
  bytes-bound; slice order = consumption order.
- level-B slots are emitted one big-group AFTER their u10 data so the PE
  (strict in-order queue) never head-of-line blocks on an in-flight
  Vector evacuation.
- level-C half 0 (obs 0..100, windows wt<=2 only) completes and DMAs out
  mid-kernel; half 1's wt3/wt4 tail runs right behind the last eps bytes.
- the measured window includes a fixed ~9us NEFF epilogue (254 semaphore
  resets + barriers) and excludes ~6us of iram loads at the front.
"""

import numpy as np
import ml_dtypes

BF16 = ml_dtypes.bfloat16

# ---------------------------------------------------------------- constants
T_TOT = 1000.0
DT = 0.1
N = 10001
TEMP_REF = 283.0
TEMP_RISE = 5.0
GAS_R = 0.008314
NSTEP = N - 1            # 10000
B = 512
NCORE = 8
BC = B // NCORE          # 64 particles per core

L1 = 10                  # level-A chunk length (steps)
NC1 = NSTEP // L1        # 1000 chunks
CPW = 5                  # chunks per window
NW = NC1 // CPW          # 200 windows
NOBS = NW + 1            # 201 observations
OBS_EVERY = 50

SUPER = 4                # chunks per level-A matmul
NMM_A = NC1 // SUPER     # 250 level-A matmuls
KE = 128                 # eps rows per level-A matmul (4 chunks x 32, 2 pad
                         # rows per chunk so blocks start 32-aligned)
NSUP_COL = 250           # eps columns groups (one per matmul)
NTILE_A = (NMM_A + 3) // 4   # 63 psum tiles (4 matmuls/tile, last has 2)

WPS = 10                 # windows per level-B slot (30 rows of 32)
NSLOT_B = NW // WPS      # 20 slots
NTILE_B = NSLOT_B // 4   # 5 u50 tiles
TAUS_PER_SLOT = 4        # u10 tiles touched per level-B slot

NOUT = 4 * NOBS          # 804
NH = NOUT // 2           # 402  (psum free-dim per matmul)

_program_cache = None
_last_results = None     # BassKernelResults of the most recent run (for test.py)

# Every psA tile is memset before use (see the loop).  The stale rows only
# ever multiply zero weight columns, BUT if a previous NEFF left NaN bits in
# PSUM, 0*NaN=NaN leaks through -- observed once as a transient all-NaN
# output.  The memsets run on the otherwise-idle Vector engine during the
# DMA-bound phase, so they are free.


# ------------------------------------------------------------- host math
def _forcings():
    times = np.linspace(0.0, T_TOT, N)
    temp = (TEMP_REF + TEMP_RISE * times / (80 * 24 * 365)
            + 10 * np.sin(2 * np.pi / 24 * times)
            + 10 * np.sin(2 * np.pi / (24 * 365) * times))
    I_S = 0.001 + 0.0005 * np.sin(2 * np.pi / (24 * 365) * times)
    I_D = 0.0001 + 5e-05 * np.sin(2 * np.pi / (24 * 365) * times)
    return temp, I_S, I_D


def _precompute(theta):
    """float64 propagator weights, packed into the device operand layouts."""
    theta = np.asarray(theta, np.float64)
    (kSr, kDr, kMr, EaS, EaD, EaM, aSD, aDS, aM, aMSC, uM, cS, cD, cM) = theta
    temp, I_S, I_D = _forcings()
    arr = lambda p, Ea: p * np.exp(-Ea / GAS_R * (1.0 / temp - 1.0 / TEMP_REF))
    k_S, k_D, k_M = arr(kSr, EaS), arr(kDr, EaD), arr(kMr, EaM)

    zeros = np.zeros(N)
    A0 = np.stack([-k_S, aDS * k_D, aM * aMSC * k_M])
    A1 = np.stack([aSD * k_S, -(uM + k_D), aM * (1 - aMSC) * k_M])
    A2 = np.stack([zeros, np.full(N, uM), -k_M])
    W = np.stack([A0, A1, A2]).transpose(2, 0, 1)          # [N,3,3]
    bias = np.stack([I_S, I_D, zeros], axis=1)             # [N,3]

    beta = np.clip(np.array([cS, cD, cM]), 1e-6, None)
    ds = np.sqrt(beta * DT)

    M = np.eye(3)[None] + DT * W[1:]                       # [10000,3,3]
    c = DT * bias[1:]                                      # [10000,3]

    # level A: within-chunk suffix products S10[c,tau] = M_{end}...M_{tau+1}
    Mc = M.reshape(NC1, L1, 3, 3)
    S10 = np.empty((NC1, L1, 3, 3))
    A10 = np.empty((NC1, 3, 3))
    for cI in range(NC1):
        acc = np.eye(3)
        S10[cI, L1 - 1] = acc
        for tau in range(L1 - 2, -1, -1):
            acc = acc @ Mc[cI, tau + 1]
            S10[cI, tau] = acc
        A10[cI] = S10[cI, 0] @ Mc[cI, 0]
    Gmat = (S10 * ds[None, None, None, :]).transpose(0, 1, 3, 2).reshape(NC1, 30, 3)

    # level B: within-window suffix products over chunks
    A10w = A10.reshape(NW, CPW, 3, 3)
    S50 = np.empty((NW, CPW, 3, 3))
    A50 = np.empty((NW, 3, 3))
    for w in range(NW):
        acc = np.eye(3)
        S50[w, CPW - 1] = acc
        for g in range(CPW - 2, -1, -1):
            acc = acc @ A10w[w, g + 1]
            S50[w, g] = acc
        A50[w] = S50[w, 0] @ A10w[w, 0]
    Hmat = S50.transpose(0, 1, 3, 2).reshape(NW, 3 * CPW, 3)   # [w, 3g+j, i]

    # deterministic trajectory at obs points (exact, float64)
    xd = np.zeros(3)
    detx = np.zeros((NOBS, 3))
    for t in range(NSTEP):
        xd = M[t] @ xd + c[t]
        if (t + 1) % OBS_EVERY == 0:
            detx[(t + 1) // OBS_EVERY] = xd

    # observation weights
    sub = np.arange(NOBS) * OBS_EVERY
    C1 = np.stack([(1 - aSD) * k_S[sub], (1 - aDS) * k_D[sub], (1 - aM) * k_M[sub]],
                  axis=1)
    Wobs = np.concatenate([np.broadcast_to(np.eye(3), (NOBS, 3, 3)),
                           C1[:, None, :]], axis=1)        # [NOBS,4,3]

    # level C: Rmat[(w,j),(n,o)] = sum_i Wobs[n,o,i] PhiW[n,w+1][i,j] (w < n)
    Rmat = np.zeros((3 * NW, NOUT))
    RX = np.zeros((3, NOUT))
    base = np.zeros(NOUT)
    for n in range(NOBS):
        WP = Wobs[n]
        base[4 * n:4 * n + 4] = WP @ detx[n]
        acc = WP.copy()
        for w in range(n - 1, -1, -1):
            Rmat[3 * w:3 * w + 3, 4 * n:4 * n + 4] = acc.T
            acc = acc @ A50[w]
        RX[:, 4 * n:4 * n + 4] = acc.T

    # ---------------- pack into device layouts (bf16) ----------------
    # Gsb [128, 3000]: block-diag lhsT shipped ready-to-use.  (On-device
    # scatter from a dense [128,750] was tried and loses: the strided
    # 3-of-12 DVE/GpSimd copies take ~3.6us each and gate every matmul.)
    # Gsb[32g+r, 12s+3g+i] = Gmat[4s+g, r, i]; rows 32g+30/31 zero pad.
    G4 = Gmat.reshape(NMM_A, SUPER, 30, 3)                 # [s,g,r,i]
    Gsb = np.zeros((SUPER, 32, NMM_A, 12), np.float32)
    for g in range(SUPER):
        Gsb[g, :30, :, 3 * g:3 * g + 3] = G4[:, g].transpose(1, 0, 2)
    Gsb = Gsb.reshape(KE, NMM_A * 12)

    # u10 row map: chunk c10, comp i -> row 32*((c10//4)%4) + 3*(c10%4) + i,
    #                                   col 64*(c10//16) + b
    # HB [128, 80*30]: level-B lhsT tiles; matmul mB = 4*slot + (tau - tau0)
    HB = np.zeros((128, NSLOT_B * TAUS_PER_SLOT * 30), np.float32)
    mB = 0
    for om in range(NSLOT_B):
        tau0 = (50 * om) // 16
        for tau in range(tau0, tau0 + TAUS_PER_SLOT):
            blk = HB[:, 30 * mB:30 * (mB + 1)]
            for rho in range(128):
                q = rho % 32
                if q >= 12:
                    continue
                c10 = 16 * tau + 4 * (rho // 32) + q // 3
                jj = q % 3
                if c10 >= NC1:
                    continue
                w = c10 // 5
                if w // WPS != om:
                    continue
                m = w - WPS * om
                g = c10 - CPW * w
                blk[rho, 3 * m:3 * m + 3] = Hmat[w, 3 * g + jj, :]
            mB += 1

    # u50 row map: window w, comp j -> row 32*((w//10)%4) + 3*(w%10) + j,
    #                                  col 64*(w//40) + b
    # Rsb: only the nonzero (triangular) column range of each (wt, half)
    # block is shipped; see _rsb_blocks() for the packing.
    blocks = _rsb_blocks()
    ncols = sum(b[3] for b in blocks)
    Rsb = np.zeros((128, ncols), np.float32)
    for wt, h, rel0, keep, off in blocks:
        blk = np.zeros((128, keep), np.float32)
        for rho in range(128):
            q = rho % 32
            if q >= 30:
                continue
            w = WPS * (4 * wt + rho // 32) + q // 3
            j = q % 3
            blk[rho] = Rmat[3 * w + j, NH * h + rel0:NH * h + rel0 + keep]
        Rsb[:, off:off + keep] = blk

    RXaug = np.concatenate([RX, base[None]], axis=0)       # [4,804]
    # hb only has data in rows 32a+q, q<12 -- ship those 48 rows densely
    HBd = HB.reshape(4, 32, -1)[:, :12].reshape(48, -1)
    return dict(Gsb=Gsb.astype(BF16), HBd=HBd.astype(BF16),
                Rsb=Rsb.astype(BF16), RXaug=RXaug.astype(BF16))


def _rsb_blocks():
    """Nonzero column ranges of each level-C (wt, half) block.

    Window-tile wt covers windows [40wt, 40wt+40); its rows only affect
    observations n >= 40wt+1, i.e. global cols >= 4*(40wt+1).  Returns
    (wt, h, rel0, keep, packed_col_offset) for each nonempty block.
    """
    blocks = []
    off = 0
    for h in range(2):
        for wt in range(NTILE_B):
            rel0 = max(0, 4 * (40 * wt + 1) - NH * h)
            if rel0 >= NH:
                continue
            keep = NH - rel0
            blocks.append((wt, h, rel0, keep, off))
            off += keep
    return blocks


def _pack_eps(noise_core):
    """[64,10000,3] f32 -> [128, 250*64]: row 32g + (3tau+j), col 64s + b =
    eps[b, t, j] for t = 10*(4s+g) + tau; rows 32g+30, 32g+31 are zero pad."""
    a = noise_core.reshape(BC, NSTEP * 3).T          # [30000, 64] view
    a = np.ascontiguousarray(a).reshape(NSUP_COL, SUPER, 30, BC)
    out = np.zeros((SUPER, 32, NSUP_COL, BC), BF16)
    out[:, :30] = a.transpose(1, 2, 0, 3).astype(BF16)
    return out.reshape(KE, NSUP_COL * BC)


# ------------------------------------------------------------ bass program
def _build_program(**bass_kwargs):
    import concourse.bass as bass
    import concourse.tile as tile
    from concourse import bacc, mybir

    f32 = mybir.dt.float32
    bf16 = mybir.dt.bfloat16
    nc = bacc.Bacc(None, target_bir_lowering=False, **bass_kwargs)

    rblocks = _rsb_blocks()
    NRSB = sum(b[3] for b in rblocks)
    HBW = NSLOT_B * TAUS_PER_SLOT * 30          # 2400 hb columns

    eps_d = nc.dram_tensor("eps", [KE, NSUP_COL * BC], bf16, kind="ExternalInput")
    gsb_d = nc.dram_tensor("gsb", [KE, NMM_A * 12], bf16, kind="ExternalInput")
    hbd_d = nc.dram_tensor("hbd", [48, HBW], bf16, kind="ExternalInput")
    rsb_d = nc.dram_tensor("rsb", [128, NRSB], bf16, kind="ExternalInput")
    xr_d = nc.dram_tensor("xr", [4, BC + NOUT], bf16, kind="ExternalInput")
    out_d = nc.dram_tensor("out", [BC, NOUT], bf16, kind="ExternalOutput")

    # eps slices aligned to level-A PSUM big-group boundaries so groups
    # unblock as their slice lands; 6 slices keeps total DMA count under the
    # HWDGE ring's outstanding-dispatch depth
    SLICES = [32, 32, 64, 64, 32, 26]
    SOFF = [0, 32, 64, 128, 192, 224, 250]

    with tile.TileContext(nc) as tc:
        with (
            tc.tile_pool(name="consts", bufs=1) as consts,
            tc.tile_pool(name="epsp", bufs=1) as epsp,
            tc.tile_pool(name="psA", bufs=2, space="PSUM") as psA,
            tc.tile_pool(name="psB", bufs=1, space="PSUM") as psB,
            tc.tile_pool(name="psC", bufs=2, space="PSUM") as psC,
        ):
            gsb = consts.tile([KE, NMM_A * 12], bf16)
            hb = consts.tile([128, HBW], bf16)
            rsb = consts.tile([128, NRSB], bf16)
            xr = consts.tile([4, BC + NOUT], bf16)
            u10 = consts.tile([128, NTILE_A * BC], bf16)
            u50 = consts.tile([128, NTILE_B * BC], bf16)
            outsb = consts.tile([BC, NOUT], bf16)
            x0t = xr[:, :BC]
            rxt = xr[:, BC:]

            eps_t = []
            for i, nsup in enumerate(SLICES):
                et = epsp.tile([KE, BC * nsup], bf16, tag=f"eps{i}")
                eps_t.append(et)

            # eps streams in-order on the Sync HWDGE ring; const weights on
            # the Scalar ring.  (Measured: moving any eps slice onto the
            # scalar ring behind the big consts, or any const onto the eps
            # ring, delays the critical eps arrivals and loses 1-3us.)
            def eps_dma(eng, i):
                eng.dma_start(out=eps_t[i],
                              in_=eps_d[:, BC * SOFF[i]:BC * SOFF[i + 1]])
            nc.gpsimd.memset(hb, 0.0)
            nc.scalar.dma_start(out=gsb, in_=gsb_d[:])
            for i in range(len(SLICES)):
                eps_dma(nc.sync, i)
            # hb data rows (32a+q, q<12) land as 4 dense row slices; the
            # zero rows come from the (otherwise idle) GpSimd memset
            for a in range(4):
                nc.scalar.dma_start(out=hb[32 * a:32 * a + 12, :],
                                    in_=hbd_d[12 * a:12 * (a + 1), :])
            nc.scalar.dma_start(out=rsb, in_=rsb_d[:])
            nc.scalar.dma_start(out=xr, in_=xr_d[:])

            def eps_rhs(s):
                for i in range(len(SLICES)):
                    if s < SOFF[i + 1]:
                        c = s - SOFF[i]
                        return eps_t[i][:, BC * c:BC * (c + 1)]
                raise AssertionError(s)

            pb = psB.tile([128, BC * NTILE_B], f32, tag="pb")
            nc.vector.memset(pb, 0.0)

            # level-B slot om becomes runnable once level-A big group
            # ((50*om+49)//16)//8 has been evacuated to u10
            def b_dep(om):
                return ((50 * om + 49) // 16) // SG_PER_TILE

            def emit_b_slot(om):
                wt, sb = om // 4, om % 4
                tau0 = (50 * om) // 16
                for ti in range(TAUS_PER_SLOT):
                    nc.tensor.matmul(
                        pb[32 * sb:32 * sb + 30, BC * wt:BC * (wt + 1)],
                        hb[:, 30 * (4 * om + ti):30 * (4 * om + ti + 1)],
                        u10[:, BC * (tau0 + ti):BC * (tau0 + ti + 1)],
                        start=(ti == 0), stop=(ti == TAUS_PER_SLOT - 1),
                        tile_position=(0, 32 * sb),
                        skip_group_check=(om != 0 or ti != 0))

            # level-C chains; blocks for wt<=2 run early (u50[:,:3*BC] ready
            # after B slot 11), the wt>=3 tail after the final B slot
            pc = [None, None]

            def emit_c_start(h):
                # h=0 only has wt<=2 blocks, so its chain closes here
                pch = psC.tile([BC, NH], f32, tag="pc")
                pc[h] = pch
                nc.tensor.matmul(pc[h], x0t, rxt[:, NH * h:NH * (h + 1)],
                                 start=True, stop=False,
                                 skip_group_check=(h != 0))
                early = [b for b in rblocks if b[1] == h and b[0] <= 2]
                for bi, (wt, _, rel0, keep, off) in enumerate(early):
                    nc.tensor.matmul(
                        pc[h][:, rel0:rel0 + keep],
                        u50[:, BC * wt:BC * (wt + 1)],
                        rsb[:, off:off + keep],
                        start=False,
                        stop=(h == 0 and bi == len(early) - 1),
                        skip_group_check=True)

            def emit_c_block(h, want_wt, stop):
                for (wt, hh, rel0, keep, off) in rblocks:
                    if wt == want_wt and hh == h:
                        nc.tensor.matmul(
                            pc[h][:, rel0:rel0 + keep],
                            u50[:, BC * wt:BC * (wt + 1)],
                            rsb[:, off:off + keep],
                            start=False, stop=stop, skip_group_check=True)

            def emit_c_out(h):
                nc.vector.tensor_copy(outsb[:, NH * h:NH * (h + 1)], pc[h])
                nc.sync.dma_start(out=out_d[:, NH * h:NH * (h + 1)],
                                  in_=outsb[:, NH * h:NH * (h + 1)])

            # ---- level A: 250 matmuls -> u10, B/C interleaved ----
            # one PSUM bank holds 8 supergroups (8 x 64 cols = 2 KB), so one
            # DVE copy evacuates 32 matmuls' worth of output
            SG_PER_TILE = 8
            next_b = 0

            def emit_a_sg(pa, sg_lo, sgA, first):
                co = BC * (sgA - sg_lo)
                nmm = 4 if sgA < NTILE_A - 1 else NMM_A - 4 * (NTILE_A - 1)
                for sig in range(nmm):
                    s = 4 * sgA + sig
                    nc.tensor.matmul(
                        pa[32 * sig:32 * sig + 12, co:co + BC],
                        gsb[:, 12 * s:12 * (s + 1)],
                        eps_rhs(s),
                        start=True, stop=True, tile_position=(0, 32 * sig),
                        # sim's group checker mis-maps offset outs
                        skip_group_check=not (first and sig == 0))

            for big in range(8):
                sg_lo = SG_PER_TILE * big
                sg_hi = min(sg_lo + SG_PER_TILE, NTILE_A)
                pa = psA.tile([128, BC * SG_PER_TILE], f32, tag="pa")
                nc.vector.memset(pa, 0.0)
                for sgA in range(sg_lo, sg_hi):
                    emit_a_sg(pa, sg_lo, sgA, sgA == sg_lo)
                nc.vector.tensor_copy(
                    u10[:, BC * sg_lo:BC * sg_hi], pa[:, :BC * (sg_hi - sg_lo)])
                # lag B slots one big-group behind their data dependency so
                # the PE never stalls on an in-flight u10 evacuation (the
                # evac runs concurrently with the next group's A stream)
                while next_b < NSLOT_B and b_dep(next_b) <= big - 1:
                    emit_b_slot(next_b)
                    next_b += 1
                    if next_b == 12:
                        # u50 for wt 0..2 + the full h=0 chain + h=1 head can
                        # run while the remaining eps still streams in
                        nc.vector.tensor_copy(u50[:, :3 * BC], pb[:, :3 * BC])
                        emit_c_start(0)
                        emit_c_start(1)
                        emit_c_out(0)

            while next_b < NSLOT_B:
                emit_b_slot(next_b)
                next_b += 1
            nc.vector.tensor_copy(u50[:, 3 * BC:], pb[:, 3 * BC:])
            emit_c_block(1, 3, stop=False)
            emit_c_block(1, 4, stop=True)
            emit_c_out(1)

    nc.finalize()
    return nc

# ------------------------------------------------------------------ kernel
def kernel(theta, x0, noise, obs_every):
    global _program_cache, _last_results
    from concourse.bass_utils import run_bass_kernel_spmd

    assert int(obs_every) == OBS_EVERY
    theta = np.asarray(theta, np.float32)
    x0 = np.asarray(x0, np.float32)
    noise = np.asarray(noise, np.float32)

    ops = _precompute(theta.astype(np.float64))

    if _program_cache is None:
        _program_cache = _build_program()
    nc = _program_cache

    in_maps = []
    for q in range(NCORE):
        sl = slice(BC * q, BC * (q + 1))
        x0aug = np.concatenate([np.ascontiguousarray(x0[sl].T),
                                np.ones((1, BC), np.float32)], axis=0)
        xr = np.concatenate([x0aug, ops["RXaug"].astype(np.float32)],
                            axis=1).astype(BF16)
        in_maps.append({
            "eps": _pack_eps(noise[sl]),
            "gsb": ops["Gsb"],
            "hbd": ops["HBd"],
            "rsb": ops["Rsb"],
            "xr": xr,
        })

    import os
    trace = bool(os.environ.get("KERNEL_TRACE"))
    res = run_bass_kernel_spmd(nc, in_maps, core_ids=list(range(NCORE)),
                               trace=trace)
    _last_results = res
    out = np.concatenate(
        [np.asarray(res.results[q]["out"]).reshape(BC, NOBS, 4)
         for q in range(NCORE)], axis=0)
    return out.astype(np.float32)

